# revision 78
# baseline (speedup 1.0000x reference)
"""MoE MLP (E=32 experts, top-2, D=H=1024) on 8 Trainium2 NeuronCores.

Strategy (expert parallel, per sharding hint):
  * Host computes the (tiny) gate: softmax(x @ Wg), top-2, renormalized
    weights, then dispatches tokens per expert into per-slot token blocks.
    Experts are rank-sorted by token count and assigned round-robin across
    cores so that slot j on every core has the same (static) capacity
    S[j] = max count in rank group j -- this cuts capacity padding from
    ~19% (uniform max capacity) to ~4%.
  * Each of the 8 cores owns 4 expert slots (weights gathered per the
    assignment) and computes GELU(x W1 + b1) W2 + b2 per slot.
  * Host combines with the top-2 gate weights (scatter-add).

Device kernel notes (from NTFF trace analysis):
  * W1 is stored e3m4 (fp8, 4 mantissa bits) x128, upconverted by the PE
    (mixed fp8xbf16 matmul); the 1/128 folds into the GELU activation's
    scale.  This halves the L1 weight traffic at ~1.4e-2 global rel err
    (vs 3.8e-3 all-bf16, gate 2e-2).  W2 stays bf16: the extra savings
    measured ~0.5us but cost 2x error margin.
  * All weight/activation HBM tensors are host-pre-tiled PARTITION-MAJOR
    ([p=128, ...contiguous...]) so every DMA is ~128 descriptors of
    2-16KB each instead of thousands of 0.6-2KB ones.
  * PE activity and DMA bandwidth share a power budget (HAM): PE-idle
    DMA runs ~430GB/s, PE-busy ~260GB/s, and a PE idle gap >~2.5us arms
    a ~10us half-speed PE window.  The schedule therefore keeps the PE
    continuously fed once compute starts and does NOT run warmup matmuls
    (they steal DMA bandwidth 1:1).
  * Trigger placement: sync+scalar are the two fast HWDGE queues; all
    x/W transfers alternate between them in consumption order, and every
    tile ring is sized so no DMA trigger ever waits (a waiting trigger
    wedges its engine queue; on scalar that blocks GELU ACTIVATEs the PE
    needs -> stall -> HAM cascade).  Next-slot transfers are issued at
    the current slot's L2 start, when scalar has no ACTIVATE work.
  * Slot 0 leads with a 128-token x chunk + per-ht 128KB W1 chunks so
    the first matmul only waits on ~0.4MB of cold DMA; its first W2
    chunk rides the otherwise-idle gpsimd queue.
  * y is written out bf16 in 2-column-tile pieces as each PSUM pair
    retires (gpsimd queue); the final pair goes as two single-tile DMAs
    on sync+scalar so the drain tail is short.
"""

import os
import sys
import numpy as np

for _p in ("/root/.axon_site/_ro/trn_rl_repo", "/opt/trn_rl_repo"):
    if _p not in sys.path and os.path.isdir(_p):
        sys.path.append(_p)

E, D, H = 32, 1024, 1024
TOP_K = 2
N_CORES = 8
EPC = E // N_CORES  # expert slots per core
ND = D // 128       # d 128-tiles
NH = H // 128       # h 128-tiles

DT_W1 = os.environ.get("MOE_DT_W1", "float8e3")
DT_W2A = os.environ.get("MOE_DT_W2A", "bfloat16")   # W2 output-cols 0:512
DT_W2B = os.environ.get("MOE_DT_W2B", "bfloat16")   # W2 output-cols 512:
DT_A = os.environ.get("MOE_DT_A", "bfloat16")
DT_Y = os.environ.get("MOE_DT_Y", "bfloat16")
NDA = ND // 2  # dt tiles in the w2a (first) half
# e3m4 weight pre-scale: W*128 fits comfortably in e3m4's +/-15.5 range
# (|W| < ~0.12); the 1/128 is folded into the activation/bias-add scale.
W1_SCALE = 128.0 if DT_W1 == "float8e3" else 1.0
W2A_SCALE = 128.0 if DT_W2A == "float8e3" else 1.0
W2B_SCALE = 128.0 if DT_W2B == "float8e3" else 1.0
N_WARMUP_MM = int(os.environ.get("MOE_WARMUP", "0"))
WU_COLS = int(os.environ.get("MOE_WU_COLS", "256"))
MAX_CW = 512  # PSUM bank limit: 512 f32 per partition

LAST_EXEC_TIME_NS = None

_NC_CACHE = {}


def _chunks(total, n):
    """Split `total` into n near-equal integer chunks."""
    base, rem = divmod(total, n)
    return [base + (1 if i < rem else 0) for i in range(n)]


def _slot_chunks(sizes):
    """Per-slot token-column chunking (shared host/device so the chunk-major
    xT/yT layouts line up).  Slot 0 gets a small leading chunk so the very
    first matmul only needs ~230KB of cold DMA; all chunks obey the PSUM
    bank limit."""
    out = []
    for j, s in enumerate(sizes):
        if j == 0 and s > 160:
            rest = s - 96
            ch = [96] + _chunks(rest, -(-rest // MAX_CW))
        else:
            ch = _chunks(s, -(-s // MAX_CW))
        out.append(ch)
    return out


def _build_nc(sizes, dt_w1_name, dt_w2a_name, dt_w2b_name, dt_a_name,
              dt_y_name):
    import concourse.bass as bass  # noqa: F401
    import concourse.tile as tile
    from concourse import bacc, mybir
    from contextlib import ExitStack

    f32 = mybir.dt.float32
    dt_w1 = getattr(mybir.dt, dt_w1_name)
    dt_w2a = getattr(mybir.dt, dt_w2a_name)
    dt_w2b = getattr(mybir.dt, dt_w2b_name)
    dt_a = getattr(mybir.dt, dt_a_name)
    dt_y = getattr(mybir.dt, dt_y_name)
    TT = sum(sizes)
    SMAX = max(sizes)

    # per-slot column-chunking (chunk-major xT/yT layout)
    slot_chunks = _slot_chunks(sizes)
    CWMAX = max(max(ch) for ch in slot_chunks)

    nc = bacc.Bacc(
        "TRN2",
        target_bir_lowering=False,
        debug=False,
        enable_asserts=False,
        num_devices=N_CORES,
    )
    # partition-major layouts: per-partition rows are contiguous in HBM
    xT = nc.dram_tensor("xT", [128, ND * TT], dt_a, kind="ExternalInput").ap()
    w1 = nc.dram_tensor("w1", [EPC, 128, NH, ND, 128], dt_w1, kind="ExternalInput").ap()
    w2a = nc.dram_tensor("w2a", [EPC, 128, NDA, NH, 128], dt_w2a,
                         kind="ExternalInput").ap()
    w2b = nc.dram_tensor("w2b", [EPC, 128, ND - NDA, NH, 128], dt_w2b,
                         kind="ExternalInput").ap()
    b1 = nc.dram_tensor("b1", [128, EPC, NH], f32, kind="ExternalInput").ap()
    b2 = nc.dram_tensor("b2", [128, EPC, ND], f32, kind="ExternalInput").ap()
    yT = nc.dram_tensor("yT", [128, ND * TT], dt_y, kind="ExternalOutput").ap()

    # W1 ht-chunks: slot 0 streams per-ht (128KB e3m4 chunks) so L1 compute
    # starts ~11us and is paced by arrivals with margin; later slots use big
    # chunks issued a half-slot (~7us) ahead of need.
    w1_chunk_plan = [[1] * NH] + [[4, 4]] * (EPC - 1)
    w2_chunk_plan = [[2, 2, 2, 2]] + [[4, 4]] * (EPC - 1)

    # ring sizing: every tile that can be in flight gets its own buffer, so
    # DMA-trigger instructions never wait on ring reuse (a waiting trigger
    # wedges its whole engine queue -> PE starvation -> HAM throttle cascade)
    xsz = ND * SMAX * mybir.dt.size(dt_a)
    xbufs = 4 if xsz <= 24 * 1024 else (2 if xsz <= 48 * 1024 else 1)
    w1_bufs = {c: sum(1 for p in w1_chunk_plan for cc in p if cc == c)
               for p in w1_chunk_plan for c in p}
    w2_bufs = {c: sum(1 for p in w2_chunk_plan for cc in p if cc == c)
               for p in w2_chunk_plan for c in p}

    with tile.TileContext(nc) as tc, ExitStack() as ctx:
        wpool = ctx.enter_context(tc.tile_pool(name="w", bufs=4))
        xpool = ctx.enter_context(tc.tile_pool(name="x", bufs=xbufs))
        hpool = ctx.enter_context(tc.tile_pool(name="h", bufs=3 * NH))
        ypool = ctx.enter_context(tc.tile_pool(name="y", bufs=4))
        bpool = ctx.enter_context(tc.tile_pool(name="b", bufs=1))
        pp1 = ctx.enter_context(tc.tile_pool(name="ps1", bufs=4, space="PSUM"))
        pp2 = ctx.enter_context(tc.tile_pool(name="ps2", bufs=3, space="PSUM"))
        ppw = ctx.enter_context(tc.tile_pool(name="psw", bufs=1, space="PSUM"))

        gelu = mybir.ActivationFunctionType.Gelu

        # --- optional warmup: ramp the PE p-state before the real matmuls.
        # NOTE: PE activity and DMA bandwidth share a power budget (HAM), so
        # warmup slows the weight stream -- default is none.
        wu = bpool.tile([128, max(WU_COLS, 128)], mybir.dt.bfloat16, tag="wu")
        nc.vector.memset(wu[:], 0.0)
        if N_WARMUP_MM:
            wups = ppw.tile([128, max(WU_COLS, 128)], f32, tag="psw")
            for i in range(N_WARMUP_MM):
                nc.tensor.matmul(wups[:, :WU_COLS], wu[:, :128], wu[:, :WU_COLS],
                                 start=(i == 0), stop=(i == N_WARMUP_MM - 1))

        # --- x/weight DMA triggers alternate sync/scalar queues (the two
        # HWDGE-capable engines).  Trigger placement is chosen so that any
        # flow-control wait on a scalar trigger never sits in front of a
        # GELU ACTIVATE that the PE is about to depend on.
        dma_engines = [nc.sync, nc.scalar]
        eng_state = [0]

        def wdma(out_ap, in_ap):
            dma_engines[eng_state[0] % 2].dma_start(out=out_ap, in_=in_ap)
            eng_state[0] += 1

        def ydma(out_ap, in_ap):
            # y writes ride the (slow but otherwise idle) gpsimd queue
            nc.gpsimd.dma_start(out=out_ap, in_=in_ap)

        xts, w1hs, w2hs = {}, {}, {}
        # chunk-major column offsets: columns of slot j chunk ci start at
        # ND * (sum of all earlier chunks' widths)
        slot_col0 = [sum(sizes[:j]) for j in range(EPC)]

        def issue_x(j):
            S = sizes[j]
            base = ND * slot_col0[j]
            xt = xpool.tile([128, ND * SMAX], dt_a, tag="xt")
            wdma(xt[:, :ND * S], xT[:, base:base + ND * S])
            xts[j] = xt

        def _w1_tile(j, ht0, cht):
            wt = wpool.tile([128, cht * ND * 128], dt_w1, tag=f"w1_{cht}",
                            bufs=w1_bufs[cht])
            return wt, (
                wt[:].rearrange("p (ht dt hi) -> p ht dt hi", ht=cht, dt=ND),
                w1[j, :, ht0:ht0 + cht],
            )

        def issue_w1(j):
            w1h = []
            ht0 = 0
            for cht in w1_chunk_plan[j]:
                wt, (o, i) = _w1_tile(j, ht0, cht)
                wdma(o, i)
                w1h.append((ht0, cht, wt))
                ht0 += cht
            w1hs[j] = w1h

        def issue_slot0_interleaved():
            """Slot 0's x and W1 spread over all THREE cold DMA queues in
            consumption order: the bytes the first matmuls need go first,
            the second x chunk rides behind the last W1 chunk."""
            S = sizes[0]
            ch = slot_chunks[0]
            cwA = ch[0]
            xt = xpool.tile([128, ND * SMAX], dt_a, tag="xt")
            xts[0] = xt
            nc.sync.dma_start(out=xt[:, :ND * cwA], in_=xT[:, :ND * cwA])
            w1h = []
            tiles = []
            for k, cht in enumerate(w1_chunk_plan[0]):
                ht0 = sum(w1_chunk_plan[0][:k])
                wt, ap = _w1_tile(0, ht0, cht)
                w1h.append((ht0, cht, wt))
                tiles.append(ap)
            w1hs[0] = w1h
            n = len(tiles)
            for k in range(n):
                eng = nc.sync if k % 2 == 0 else nc.scalar
                eng.dma_start(out=tiles[k][0], in_=tiles[k][1])
                if k == min(3, n - 1) and len(ch) > 1:
                    # second x chunk after the first few W1 chunks
                    nc.scalar.dma_start(out=xt[:, ND * cwA:ND * S],
                                        in_=xT[:, ND * cwA:ND * S])

        def issue_w2(j, lo, hi, eng=None):
            w2h = w2hs.setdefault(j, [])
            dt0 = sum(w2_chunk_plan[j][:lo])
            for cdt in w2_chunk_plan[j][lo:hi]:
                # chunks never straddle the w2a/w2b precision boundary
                if dt0 < NDA:
                    src, loc, dt_w2c, half = w2a, dt0, dt_w2a, "a"
                else:
                    src, loc, dt_w2c, half = w2b, dt0 - NDA, dt_w2b, "b"
                wt = wpool.tile([128, cdt * NH * 128], dt_w2c,
                                tag=f"w2{half}_{cdt}",
                                bufs=max(1, w2_bufs[cdt] // 2))
                out_ap = wt[:].rearrange("p (dt ht di) -> p dt ht di",
                                         dt=cdt, ht=NH)
                in_ap = src[j, :, loc:loc + cdt]
                if eng is not None:
                    eng.dma_start(out=out_ap, in_=in_ap)
                else:
                    wdma(out_ap, in_ap)
                w2h.append((dt0, cdt, wt))
                dt0 += cdt

        # biases ride the gpsimd queue (cold-parallel; they are only needed
        # by the first ACTIVATE, ~1us after the first matmul), keeping the
        # two fast queues free for the latency-critical x/W1 stream
        b1_sb = bpool.tile([128, EPC * NH], f32, tag="b1")
        b2_sb = bpool.tile([128, EPC * ND], f32, tag="b2")
        nc.gpsimd.dma_start(
            out=b1_sb[:].rearrange("p (e ht) -> p e ht", e=EPC), in_=b1[:])
        nc.gpsimd.dma_start(
            out=b2_sb[:].rearrange("p (e dt) -> p e dt", e=EPC), in_=b2[:])
        # slot 0: x + all W1 up front on the fast queues; first W2 chunk on
        # the slow-but-idle gpsimd queue (needed only when L2 starts); the
        # remaining W2 chunks follow behind the first ACTIVATE
        issue_slot0_interleaved()
        issue_w2(0, 0, 1, eng=nc.gpsimd)
        # pre-load the GELU table on the scalar engine off the critical
        # path (after scalar's first DMA triggers so they go out first)
        wug = bpool.tile([128, 1], f32, tag="wug")
        nc.scalar.activation(wug[:], wu[:, :1], gelu)

        for j in range(EPC):
            xt, w1h = xts[j], w1hs[j]
            ch_list = slot_chunks[j]
            # ---- layer 1 over all chunks: h = gelu((x W1)/s + b1)
            hts_all = []
            xoff = 0
            for ci, cw in enumerate(ch_list):
                hts = []
                for ht in range(NH):
                    ht0, cht, wt = next(w for w in w1h if w[0] <= ht < w[0] + w[1])
                    hoff = ((ht - ht0) * ND) * 128
                    ps = pp1.tile([128, CWMAX], f32, tag="ps1")
                    for dt_i in range(ND):
                        nc.tensor.matmul(
                            ps[:, :cw],
                            wt[:, hoff + dt_i * 128: hoff + (dt_i + 1) * 128],
                            xt[:, xoff + dt_i * cw: xoff + (dt_i + 1) * cw],
                            start=(dt_i == 0),
                            stop=(dt_i == ND - 1),
                        )
                    hsb = hpool.tile([128, CWMAX], dt_a, tag="ht")
                    nc.scalar.activation(
                        hsb[:, :cw], ps[:, :cw], gelu,
                        bias=b1_sb[:, j * NH + ht: j * NH + ht + 1],
                        scale=1.0 / W1_SCALE,
                    )
                    hts.append(hsb)
                    if j == 0 and ci == 0 and ht == 0:
                        # rest of slot 0's W2 behind the first ACTIVATE
                        issue_w2(0, 1, len(w2_chunk_plan[0]))
                hts_all.append(hts)
                xoff += ND * cw
            # ---- next slot's transfers before the L2 phase (the scalar
            # engine has no ACTIVATE work during L2, so a flow-control wait
            # on its triggers cannot block anything the PE needs soon)
            if j + 1 < EPC:
                issue_x(j + 1)
                issue_w1(j + 1)
                issue_w2(j + 1, 0, len(w2_chunk_plan[j + 1]))
            # ---- layer 2 over all chunks: y = (h W2)/s + b2
            w2h = w2hs[j]
            colbase = slot_col0[j]
            for ci, cw in enumerate(ch_list):
                ybase = ND * colbase
                hts = hts_all[ci]
                ysb = ypool.tile([128, ND * CWMAX], dt_y, tag="yt")
                for dt_i in range(ND):
                    dt0, cdt, wt = next(w for w in w2h if w[0] <= dt_i < w[0] + w[1])
                    doff = ((dt_i - dt0) * NH) * 128
                    ps2 = pp2.tile([128, CWMAX], f32, tag="ps2")
                    for ht in range(NH):
                        nc.tensor.matmul(
                            ps2[:, :cw],
                            wt[:, doff + ht * 128: doff + (ht + 1) * 128],
                            hts[ht][:, :cw],
                            start=(ht == 0),
                            stop=(ht == NH - 1),
                        )
                    w2scale = W2A_SCALE if dt_i < NDA else W2B_SCALE
                    if w2scale != 1.0:
                        nc.vector.tensor_scalar(
                            ysb[:, dt_i * cw:(dt_i + 1) * cw], ps2[:, :cw],
                            1.0 / w2scale,
                            b2_sb[:, j * ND + dt_i: j * ND + dt_i + 1],
                            op0=mybir.AluOpType.mult,
                            op1=mybir.AluOpType.add,
                        )
                    else:
                        nc.vector.tensor_scalar_add(
                            ysb[:, dt_i * cw:(dt_i + 1) * cw], ps2[:, :cw],
                            b2_sb[:, j * ND + dt_i: j * ND + dt_i + 1],
                        )
                    last_pair = (j == EPC - 1 and dt_i == ND - 1
                                 and ci == len(ch_list) - 1)
                    if dt_i % 2 == 1 and not last_pair:
                        # stream out this pair of column tiles (contiguous
                        # in the chunk-major layout: one descriptor/partition)
                        ydma(
                            yT[:, ybase + (dt_i - 1) * cw:
                               ybase + (dt_i + 1) * cw],
                            ysb[:, (dt_i - 1) * cw:(dt_i + 1) * cw],
                        )
                    elif last_pair:
                        # final pair: two single-tile DMAs on the two fast
                        # queues in parallel to shorten the drain tail
                        nc.scalar.dma_start(
                            out=yT[:, ybase + (dt_i - 1) * cw:
                                   ybase + dt_i * cw],
                            in_=ysb[:, (dt_i - 1) * cw:dt_i * cw],
                        )
                        nc.sync.dma_start(
                            out=yT[:, ybase + dt_i * cw:
                                   ybase + (dt_i + 1) * cw],
                            in_=ysb[:, dt_i * cw:(dt_i + 1) * cw],
                        )
                colbase += cw
    nc.compile()
    return nc


def _get_nc(sizes, dt_w1, dt_w2a, dt_w2b, dt_a, dt_y):
    key = (tuple(sizes), dt_w1, dt_w2a, dt_w2b, dt_a, dt_y)
    if key not in _NC_CACHE:
        _NC_CACHE[key] = _build_nc(list(sizes), dt_w1, dt_w2a, dt_w2b,
                                   dt_a, dt_y)
    return _NC_CACHE[key]


def _np_dt(name):
    import ml_dtypes
    if name == "float32":
        return np.dtype(np.float32)
    if name == "float8e3":
        return np.dtype(ml_dtypes.float8_e3m4)
    return np.dtype(getattr(ml_dtypes, name))


def _route(xf, Wg):
    """Replicates the reference gate exactly in f32 numpy."""
    logits = xf @ Wg                                     # [T, E]
    m = logits.max(-1, keepdims=True)
    ex = np.exp(logits - m)
    scores = ex / ex.sum(-1, keepdims=True)
    idx = np.argsort(-scores, axis=1, kind="stable")[:, :TOP_K]  # [T, k]
    tw = np.take_along_axis(scores, idx, 1)
    m2 = tw.max(-1, keepdims=True)
    e2 = np.exp(tw - m2)
    w = (e2 / e2.sum(-1, keepdims=True)).astype(np.float32)
    return idx.astype(np.int64), w


def kernel(x, Wg, W1, b1, W2, b2):
    global LAST_EXEC_TIME_NS
    from concourse import bass_utils

    dt_w1, dt_w2a, dt_w2b = DT_W1, DT_W2A, DT_W2B
    dt_a, dt_y = DT_A, DT_Y
    orig_shape = x.shape
    x = np.asarray(x, dtype=np.float32)
    Wg = np.asarray(Wg, dtype=np.float32)
    W1 = np.asarray(W1, dtype=np.float32)
    b1 = np.asarray(b1, dtype=np.float32)
    W2 = np.asarray(W2, dtype=np.float32)
    b2 = np.asarray(b2, dtype=np.float32)
    xf = np.ascontiguousarray(x.reshape(-1, D))
    T = xf.shape[0]

    idx, w = _route(xf, Wg)

    # ---- slot assignment: rank experts by count, group ranks of N_CORES,
    # slot j capacity = max count in group j (aligned up to 4)
    counts = np.bincount(idx.reshape(-1), minlength=E)
    order_e = np.argsort(-counts, kind="stable")         # expert ids by rank
    exp_core = np.empty(E, np.int64)
    exp_slot = np.empty(E, np.int64)
    sizes = []
    for j in range(EPC):
        grp = order_e[j * N_CORES:(j + 1) * N_CORES]
        exp_core[grp] = np.arange(N_CORES)
        exp_slot[grp] = j
        sizes.append(max(4, int(-(-int(counts[grp].max()) // 4) * 4)))
    TT = sum(sizes)
    slot_off = np.concatenate([[0], np.cumsum(sizes)])   # token offsets per slot

    # ---- dispatch: sort assignments by expert, position within expert
    flat_e = idx.reshape(-1)                 # [T*k]
    flat_t = np.repeat(np.arange(T), TOP_K)
    sorder = np.argsort(flat_e, kind="stable")
    starts = np.zeros(E + 1, np.int64)
    starts[1:] = np.cumsum(counts)
    se = flat_e[sorder]
    pos = np.arange(TOP_K * T) - starts[se]
    core = exp_core[se]
    slot = exp_slot[se]
    tok = flat_t[sorder]

    # token id occupying each (core, slot, pos); padding -> token 0
    gidx = np.zeros((N_CORES, TT), np.int64)
    for c in range(N_CORES):
        msel = core == c
        gidx[c, slot_off[slot[msel]] + pos[msel]] = tok[msel]

    np_w1 = _np_dt(dt_w1)
    np_w2a = _np_dt(dt_w2a)
    np_w2b = _np_dt(dt_w2b)
    np_a = _np_dt(dt_a)
    xf_a = xf.astype(np_a, copy=False)
    # pre-tile weights partition-major:
    #   w1[e] = [p(=d_in%128), ht, dt(=d_in//128), hi]
    #   w2[e] = [p(=h_in%128), dt, ht(=h_in//128), di]  (split at dt=NDA)
    W1t = np.ascontiguousarray(
        (W1 * W1_SCALE).reshape(E, ND, 128, NH, 128)
        .transpose(0, 2, 3, 1, 4).astype(np_w1, copy=False))
    W2r = W2.reshape(E, NH, 128, ND, 128).transpose(0, 2, 3, 1, 4)
    W2ta = np.ascontiguousarray(
        (W2r[:, :, :NDA] * W2A_SCALE).astype(np_w2a, copy=False))
    W2tb = np.ascontiguousarray(
        (W2r[:, :, NDA:] * W2B_SCALE).astype(np_w2b, copy=False))
    b1t = np.ascontiguousarray(b1.reshape(E, NH, 128).transpose(2, 0, 1))
    b2t = np.ascontiguousarray(b2.reshape(E, ND, 128).transpose(2, 0, 1))

    # chunk-major (j, ci) column blocks; must match the device layout
    chunks = _slot_chunks(sizes)
    blocks = []          # (slot j, token offset within slot, width, colbase)
    colbase = 0
    for j in range(EPC):
        c0 = 0
        for cw in chunks[j]:
            blocks.append((j, c0, cw, colbase))
            c0 += cw
            colbase += cw

    in_maps = []
    for c in range(N_CORES):
        sl_experts = np.array(
            [order_e[j * N_CORES + c] for j in range(EPC)], np.int64)
        # xT: [128, ND*TT]; chunk block = [128, ND, cw] = x[tok, dt*128+p]
        xTc = np.zeros((128, ND * TT), np_a)
        for j, c0, cw, cb in blocks:
            blk = xf_a[gidx[c, slot_off[j] + c0: slot_off[j] + c0 + cw]]
            xTc[:, ND * cb: ND * (cb + cw)] = (
                blk.reshape(cw, ND, 128).transpose(2, 1, 0).reshape(128, ND * cw))
        in_maps.append({
            "xT": np.ascontiguousarray(xTc),
            "w1": W1t[sl_experts],
            "w2a": W2ta[sl_experts],
            "w2b": W2tb[sl_experts],
            "b1": np.ascontiguousarray(b1t[:, sl_experts]),
            "b2": np.ascontiguousarray(b2t[:, sl_experts]),
        })

    nc = _get_nc(sizes, dt_w1, dt_w2a, dt_w2b, dt_a, dt_y)
    trace = os.environ.get("MOE_TRACE", "0") == "1"
    res = bass_utils.run_bass_kernel_spmd(
        nc, in_maps, core_ids=list(range(N_CORES)), trace=trace,
    )
    LAST_EXEC_TIME_NS = res.exec_time_ns

    # ---- combine: y[t] += w * yT[core][:, chunk block][:, :, pos]
    Y = np.zeros((N_CORES, TT, D), np.float32)
    for c in range(N_CORES):
        yTc = np.asarray(res.results[c]["yT"], dtype=np.float32)
        for j, c0, cw, cb in blocks:
            blk = yTc[:, ND * cb: ND * (cb + cw)]
            Y[c, slot_off[j] + c0: slot_off[j] + c0 + cw] = (
                blk.reshape(128, ND, cw).transpose(2, 1, 0).reshape(cw, D))

    contrib = Y[core, slot_off[slot] + pos]  # [T*k, D] in sorted order
    inv = np.empty_like(sorder)
    inv[sorder] = np.arange(TOP_K * T)
    contrib = contrib[inv].reshape(T, TOP_K, D)
    y = (contrib * w[:, :, None]).sum(1).astype(np.float32)
    return y.reshape(orig_shape)


# revision 80
# speedup vs baseline: 1.0335x; 1.0335x over previous
"""MoE MLP (E=32 experts, top-2, D=H=1024) on 8 Trainium2 NeuronCores.

Strategy (expert parallel, per sharding hint):
  * Host computes the (tiny) gate: softmax(x @ Wg), top-2, renormalized
    weights, then dispatches tokens per expert into per-slot token blocks.
    Experts are rank-sorted by token count and assigned round-robin across
    cores so that slot j on every core has the same (static) capacity
    S[j] = max count in rank group j -- this cuts capacity padding from
    ~19% (uniform max capacity) to ~4%.
  * Each of the 8 cores owns 4 expert slots (weights gathered per the
    assignment) and computes GELU(x W1 + b1) W2 + b2 per slot.
  * Host combines with the top-2 gate weights (scatter-add).

Device kernel notes (from NTFF trace analysis):
  * W1 is stored e3m4 (fp8, 4 mantissa bits) x128, upconverted by the PE
    (mixed fp8xbf16 matmul); the 1/128 folds into the GELU activation's
    scale.  This halves the L1 weight traffic at ~1.4e-2 global rel err
    (vs 3.8e-3 all-bf16, gate 2e-2).  W2 stays bf16: the extra savings
    measured ~0.5us but cost 2x error margin.
  * All weight/activation HBM tensors are host-pre-tiled PARTITION-MAJOR
    ([p=128, ...contiguous...]) so every DMA is ~128 descriptors of
    2-16KB each instead of thousands of 0.6-2KB ones.
  * PE activity and DMA bandwidth share a power budget (HAM): PE-idle
    DMA runs ~430GB/s, PE-busy ~260GB/s, and a PE idle gap >~2.5us arms
    a ~10us half-speed PE window.  The schedule therefore keeps the PE
    continuously fed once compute starts and does NOT run warmup matmuls
    (they steal DMA bandwidth 1:1).
  * Trigger placement: sync+scalar are the two fast HWDGE queues; all
    x/W transfers alternate between them in consumption order, and every
    tile ring is sized so no DMA trigger ever waits (a waiting trigger
    wedges its engine queue; on scalar that blocks GELU ACTIVATEs the PE
    needs -> stall -> HAM cascade).  Next-slot transfers are issued at
    the current slot's L2 start, when scalar has no ACTIVATE work.
  * Slot 0 leads with a 128-token x chunk + per-ht 128KB W1 chunks so
    the first matmul only waits on ~0.4MB of cold DMA; its first W2
    chunk rides the otherwise-idle gpsimd queue.
  * y is written out bf16 in 2-column-tile pieces as each PSUM pair
    retires (gpsimd queue); the final pair goes as two single-tile DMAs
    on sync+scalar so the drain tail is short.
"""

import os
import sys
import numpy as np

for _p in ("/root/.axon_site/_ro/trn_rl_repo", "/opt/trn_rl_repo"):
    if _p not in sys.path and os.path.isdir(_p):
        sys.path.append(_p)

E, D, H = 32, 1024, 1024
TOP_K = 2
N_CORES = 8
EPC = E // N_CORES  # expert slots per core
ND = D // 128       # d 128-tiles
NH = H // 128       # h 128-tiles

DT_W1 = os.environ.get("MOE_DT_W1", "float8e3")
DT_W2A = os.environ.get("MOE_DT_W2A", "bfloat16")   # W2 output-cols 0:512
DT_W2B = os.environ.get("MOE_DT_W2B", "bfloat16")   # W2 output-cols 512:
DT_A = os.environ.get("MOE_DT_A", "bfloat16")
DT_Y = os.environ.get("MOE_DT_Y", "bfloat16")
NDA = ND // 2  # dt tiles in the w2a (first) half
# e3m4 weight pre-scale: W*128 fits comfortably in e3m4's +/-15.5 range
# (|W| < ~0.12); the 1/128 is folded into the activation/bias-add scale.
W1_SCALE = 128.0 if DT_W1 == "float8e3" else 1.0
W2A_SCALE = 128.0 if DT_W2A == "float8e3" else 1.0
W2B_SCALE = 128.0 if DT_W2B == "float8e3" else 1.0
N_WARMUP_MM = int(os.environ.get("MOE_WARMUP", "0"))
WU_COLS = int(os.environ.get("MOE_WU_COLS", "256"))
MAX_CW = 512  # PSUM bank limit: 512 f32 per partition

LAST_EXEC_TIME_NS = None

_NC_CACHE = {}


def _chunks(total, n):
    """Split `total` into n near-equal integer chunks."""
    base, rem = divmod(total, n)
    return [base + (1 if i < rem else 0) for i in range(n)]


def _slot_chunks(sizes):
    """Per-slot token-column chunking (shared host/device so the chunk-major
    xT/yT layouts line up).  Slot 0 gets a small leading chunk so the very
    first matmul only needs ~230KB of cold DMA; all chunks obey the PSUM
    bank limit."""
    out = []
    for j, s in enumerate(sizes):
        if j == 0 and s > 192:
            rest = s - 128
            ch = [128] + _chunks(rest, -(-rest // MAX_CW))
        else:
            ch = _chunks(s, -(-s // MAX_CW))
        out.append(ch)
    return out


def _build_nc(sizes, dt_w1_name, dt_w2a_name, dt_w2b_name, dt_a_name,
              dt_y_name):
    import concourse.bass as bass  # noqa: F401
    import concourse.tile as tile
    from concourse import bacc, mybir
    from contextlib import ExitStack

    f32 = mybir.dt.float32
    dt_w1 = getattr(mybir.dt, dt_w1_name)
    dt_w2a = getattr(mybir.dt, dt_w2a_name)
    dt_w2b = getattr(mybir.dt, dt_w2b_name)
    dt_a = getattr(mybir.dt, dt_a_name)
    dt_y = getattr(mybir.dt, dt_y_name)
    TT = sum(sizes)
    SMAX = max(sizes)

    # per-slot column-chunking (chunk-major xT/yT layout)
    slot_chunks = _slot_chunks(sizes)
    CWMAX = max(max(ch) for ch in slot_chunks)

    nc = bacc.Bacc(
        "TRN2",
        target_bir_lowering=False,
        debug=False,
        enable_asserts=False,
        num_devices=N_CORES,
    )
    # partition-major layouts: per-partition rows are contiguous in HBM
    xT = nc.dram_tensor("xT", [128, ND * TT], dt_a, kind="ExternalInput").ap()
    w1 = nc.dram_tensor("w1", [EPC, 128, NH, ND, 128], dt_w1, kind="ExternalInput").ap()
    w2a = nc.dram_tensor("w2a", [EPC, 128, NDA, NH, 128], dt_w2a,
                         kind="ExternalInput").ap()
    w2b = nc.dram_tensor("w2b", [EPC, 128, ND - NDA, NH, 128], dt_w2b,
                         kind="ExternalInput").ap()
    b1 = nc.dram_tensor("b1", [128, EPC, NH], f32, kind="ExternalInput").ap()
    b2 = nc.dram_tensor("b2", [128, EPC, ND], f32, kind="ExternalInput").ap()
    yT = nc.dram_tensor("yT", [128, ND * TT], dt_y, kind="ExternalOutput").ap()

    # W1 ht-chunks: slot 0 streams per-ht (128KB e3m4 chunks) so L1 compute
    # starts ~11us and is paced by arrivals with margin; later slots use big
    # chunks issued a half-slot (~7us) ahead of need.
    w1_chunk_plan = [[1] * NH] + [[4, 4]] * (EPC - 1)
    w2_chunk_plan = [[2, 2, 2, 2]] + [[4, 4]] * (EPC - 1)

    # ring sizing: every tile that can be in flight gets its own buffer, so
    # DMA-trigger instructions never wait on ring reuse (a waiting trigger
    # wedges its whole engine queue -> PE starvation -> HAM throttle cascade)
    xsz = ND * SMAX * mybir.dt.size(dt_a)
    xbufs = 4 if xsz <= 24 * 1024 else (2 if xsz <= 48 * 1024 else 1)
    w1_bufs = {c: sum(1 for p in w1_chunk_plan for cc in p if cc == c)
               for p in w1_chunk_plan for c in p}
    w2_bufs = {c: sum(1 for p in w2_chunk_plan for cc in p if cc == c)
               for p in w2_chunk_plan for c in p}

    with tile.TileContext(nc) as tc, ExitStack() as ctx:
        wpool = ctx.enter_context(tc.tile_pool(name="w", bufs=4))
        xpool = ctx.enter_context(tc.tile_pool(name="x", bufs=xbufs))
        hpool = ctx.enter_context(tc.tile_pool(name="h", bufs=3 * NH))
        ypool = ctx.enter_context(tc.tile_pool(name="y", bufs=4))
        bpool = ctx.enter_context(tc.tile_pool(name="b", bufs=1))
        pp1 = ctx.enter_context(tc.tile_pool(name="ps1", bufs=4, space="PSUM"))
        pp2 = ctx.enter_context(tc.tile_pool(name="ps2", bufs=3, space="PSUM"))
        ppw = ctx.enter_context(tc.tile_pool(name="psw", bufs=1, space="PSUM"))

        gelu = mybir.ActivationFunctionType.Gelu

        # --- optional warmup: ramp the PE p-state before the real matmuls.
        # NOTE: PE activity and DMA bandwidth share a power budget (HAM), so
        # warmup slows the weight stream -- default is none.
        wu = bpool.tile([128, max(WU_COLS, 128)], mybir.dt.bfloat16, tag="wu")
        nc.vector.memset(wu[:], 0.0)
        if N_WARMUP_MM:
            wups = ppw.tile([128, max(WU_COLS, 128)], f32, tag="psw")
            for i in range(N_WARMUP_MM):
                nc.tensor.matmul(wups[:, :WU_COLS], wu[:, :128], wu[:, :WU_COLS],
                                 start=(i == 0), stop=(i == N_WARMUP_MM - 1))

        # --- x/weight DMA triggers alternate sync/scalar queues (the two
        # HWDGE-capable engines).  Trigger placement is chosen so that any
        # flow-control wait on a scalar trigger never sits in front of a
        # GELU ACTIVATE that the PE is about to depend on.
        dma_engines = [nc.sync, nc.scalar]
        eng_state = [0]

        def wdma(out_ap, in_ap):
            dma_engines[eng_state[0] % 2].dma_start(out=out_ap, in_=in_ap)
            eng_state[0] += 1

        def ydma(out_ap, in_ap):
            # y writes ride the (slow but otherwise idle) gpsimd queue
            nc.gpsimd.dma_start(out=out_ap, in_=in_ap)

        xts, w1hs, w2hs = {}, {}, {}
        # chunk-major column offsets: columns of slot j chunk ci start at
        # ND * (sum of all earlier chunks' widths)
        slot_col0 = [sum(sizes[:j]) for j in range(EPC)]

        def issue_x(j):
            S = sizes[j]
            base = ND * slot_col0[j]
            xt = xpool.tile([128, ND * SMAX], dt_a, tag="xt")
            wdma(xt[:, :ND * S], xT[:, base:base + ND * S])
            xts[j] = xt

        def _w1_tile(j, ht0, cht):
            wt = wpool.tile([128, cht * ND * 128], dt_w1, tag=f"w1_{cht}",
                            bufs=w1_bufs[cht])
            return wt, (
                wt[:].rearrange("p (ht dt hi) -> p ht dt hi", ht=cht, dt=ND),
                w1[j, :, ht0:ht0 + cht],
            )

        def issue_w1(j):
            w1h = []
            ht0 = 0
            for cht in w1_chunk_plan[j]:
                wt, (o, i) = _w1_tile(j, ht0, cht)
                wdma(o, i)
                w1h.append((ht0, cht, wt))
                ht0 += cht
            w1hs[j] = w1h

        def issue_slot0_interleaved():
            """Slot 0's x and W1 spread over all THREE cold DMA queues in
            consumption order: the bytes the first matmuls need go first,
            the second x chunk rides behind the last W1 chunk."""
            S = sizes[0]
            ch = slot_chunks[0]
            cwA = ch[0]
            xt = xpool.tile([128, ND * SMAX], dt_a, tag="xt")
            xts[0] = xt
            nc.sync.dma_start(out=xt[:, :ND * cwA], in_=xT[:, :ND * cwA])
            w1h = []
            tiles = []
            for k, cht in enumerate(w1_chunk_plan[0]):
                ht0 = sum(w1_chunk_plan[0][:k])
                wt, ap = _w1_tile(0, ht0, cht)
                w1h.append((ht0, cht, wt))
                tiles.append(ap)
            w1hs[0] = w1h
            n = len(tiles)
            for k in range(n):
                eng = nc.sync if k % 2 == 0 else nc.scalar
                eng.dma_start(out=tiles[k][0], in_=tiles[k][1])
                if k == min(3, n - 1) and len(ch) > 1:
                    # second x chunk after the first few W1 chunks
                    nc.scalar.dma_start(out=xt[:, ND * cwA:ND * S],
                                        in_=xT[:, ND * cwA:ND * S])

        def issue_w2(j, lo, hi, eng=None):
            w2h = w2hs.setdefault(j, [])
            dt0 = sum(w2_chunk_plan[j][:lo])
            for cdt in w2_chunk_plan[j][lo:hi]:
                # chunks never straddle the w2a/w2b precision boundary
                if dt0 < NDA:
                    src, loc, dt_w2c, half = w2a, dt0, dt_w2a, "a"
                else:
                    src, loc, dt_w2c, half = w2b, dt0 - NDA, dt_w2b, "b"
                wt = wpool.tile([128, cdt * NH * 128], dt_w2c,
                                tag=f"w2{half}_{cdt}",
                                bufs=max(1, w2_bufs[cdt] // 2))
                out_ap = wt[:].rearrange("p (dt ht di) -> p dt ht di",
                                         dt=cdt, ht=NH)
                in_ap = src[j, :, loc:loc + cdt]
                if eng is not None:
                    eng.dma_start(out=out_ap, in_=in_ap)
                else:
                    wdma(out_ap, in_ap)
                w2h.append((dt0, cdt, wt))
                dt0 += cdt

        # biases go out first on the two fast queues: tiny transfers that
        # warm the DGE rings before the latency-critical x/W1 stream
        b1_sb = bpool.tile([128, EPC * NH], f32, tag="b1")
        b2_sb = bpool.tile([128, EPC * ND], f32, tag="b2")
        nc.sync.dma_start(
            out=b1_sb[:].rearrange("p (e ht) -> p e ht", e=EPC), in_=b1[:])
        nc.scalar.dma_start(
            out=b2_sb[:].rearrange("p (e dt) -> p e dt", e=EPC), in_=b2[:])
        # slot 0: x + all W1 up front on the fast queues; first W2 chunk on
        # the slow-but-idle gpsimd queue (needed only when L2 starts); the
        # remaining W2 chunks follow behind the first ACTIVATE
        issue_slot0_interleaved()
        issue_w2(0, 0, 1, eng=nc.gpsimd)
        # pre-load the GELU table on the scalar engine off the critical
        # path (after scalar's first DMA triggers so they go out first)
        wug = bpool.tile([128, 1], f32, tag="wug")
        nc.scalar.activation(wug[:], wu[:, :1], gelu)

        for j in range(EPC):
            xt, w1h = xts[j], w1hs[j]
            ch_list = slot_chunks[j]
            # ---- layer 1 over all chunks: h = gelu((x W1)/s + b1)
            hts_all = []
            xoff = 0
            for ci, cw in enumerate(ch_list):
                hts = []
                for ht in range(NH):
                    ht0, cht, wt = next(w for w in w1h if w[0] <= ht < w[0] + w[1])
                    hoff = ((ht - ht0) * ND) * 128
                    ps = pp1.tile([128, CWMAX], f32, tag="ps1")
                    for dt_i in range(ND):
                        nc.tensor.matmul(
                            ps[:, :cw],
                            wt[:, hoff + dt_i * 128: hoff + (dt_i + 1) * 128],
                            xt[:, xoff + dt_i * cw: xoff + (dt_i + 1) * cw],
                            start=(dt_i == 0),
                            stop=(dt_i == ND - 1),
                        )
                    hsb = hpool.tile([128, CWMAX], dt_a, tag="ht")
                    nc.scalar.activation(
                        hsb[:, :cw], ps[:, :cw], gelu,
                        bias=b1_sb[:, j * NH + ht: j * NH + ht + 1],
                        scale=1.0 / W1_SCALE,
                    )
                    hts.append(hsb)
                    if j == 0 and ci == 0 and ht == 0:
                        # rest of slot 0's W2 behind the first ACTIVATE
                        issue_w2(0, 1, len(w2_chunk_plan[0]))
                hts_all.append(hts)
                xoff += ND * cw
            # ---- next slot's transfers before the L2 phase (the scalar
            # engine has no ACTIVATE work during L2, so a flow-control wait
            # on its triggers cannot block anything the PE needs soon)
            if j + 1 < EPC:
                issue_x(j + 1)
                issue_w1(j + 1)
                issue_w2(j + 1, 0, len(w2_chunk_plan[j + 1]))
            # ---- layer 2 over all chunks: y = (h W2)/s + b2
            w2h = w2hs[j]
            colbase = slot_col0[j]
            for ci, cw in enumerate(ch_list):
                ybase = ND * colbase
                hts = hts_all[ci]
                ysb = ypool.tile([128, ND * CWMAX], dt_y, tag="yt")
                for dt_i in range(ND):
                    dt0, cdt, wt = next(w for w in w2h if w[0] <= dt_i < w[0] + w[1])
                    doff = ((dt_i - dt0) * NH) * 128
                    ps2 = pp2.tile([128, CWMAX], f32, tag="ps2")
                    for ht in range(NH):
                        nc.tensor.matmul(
                            ps2[:, :cw],
                            wt[:, doff + ht * 128: doff + (ht + 1) * 128],
                            hts[ht][:, :cw],
                            start=(ht == 0),
                            stop=(ht == NH - 1),
                        )
                    w2scale = W2A_SCALE if dt_i < NDA else W2B_SCALE
                    if w2scale != 1.0:
                        nc.vector.tensor_scalar(
                            ysb[:, dt_i * cw:(dt_i + 1) * cw], ps2[:, :cw],
                            1.0 / w2scale,
                            b2_sb[:, j * ND + dt_i: j * ND + dt_i + 1],
                            op0=mybir.AluOpType.mult,
                            op1=mybir.AluOpType.add,
                        )
                    else:
                        nc.vector.tensor_scalar_add(
                            ysb[:, dt_i * cw:(dt_i + 1) * cw], ps2[:, :cw],
                            b2_sb[:, j * ND + dt_i: j * ND + dt_i + 1],
                        )
                    last_pair = (j == EPC - 1 and dt_i == ND - 1
                                 and ci == len(ch_list) - 1)
                    if dt_i % 2 == 1 and not last_pair:
                        # stream out this pair of column tiles (contiguous
                        # in the chunk-major layout: one descriptor/partition)
                        ydma(
                            yT[:, ybase + (dt_i - 1) * cw:
                               ybase + (dt_i + 1) * cw],
                            ysb[:, (dt_i - 1) * cw:(dt_i + 1) * cw],
                        )
                    elif last_pair:
                        # final pair: two single-tile DMAs on the two fast
                        # queues in parallel to shorten the drain tail
                        nc.scalar.dma_start(
                            out=yT[:, ybase + (dt_i - 1) * cw:
                                   ybase + dt_i * cw],
                            in_=ysb[:, (dt_i - 1) * cw:dt_i * cw],
                        )
                        nc.sync.dma_start(
                            out=yT[:, ybase + dt_i * cw:
                                   ybase + (dt_i + 1) * cw],
                            in_=ysb[:, dt_i * cw:(dt_i + 1) * cw],
                        )
                colbase += cw
    nc.compile()
    return nc


def _get_nc(sizes, dt_w1, dt_w2a, dt_w2b, dt_a, dt_y):
    key = (tuple(sizes), dt_w1, dt_w2a, dt_w2b, dt_a, dt_y)
    if key not in _NC_CACHE:
        _NC_CACHE[key] = _build_nc(list(sizes), dt_w1, dt_w2a, dt_w2b,
                                   dt_a, dt_y)
    return _NC_CACHE[key]


def _np_dt(name):
    import ml_dtypes
    if name == "float32":
        return np.dtype(np.float32)
    if name == "float8e3":
        return np.dtype(ml_dtypes.float8_e3m4)
    return np.dtype(getattr(ml_dtypes, name))


def _route(xf, Wg):
    """Replicates the reference gate exactly in f32 numpy."""
    logits = xf @ Wg                                     # [T, E]
    m = logits.max(-1, keepdims=True)
    ex = np.exp(logits - m)
    scores = ex / ex.sum(-1, keepdims=True)
    idx = np.argsort(-scores, axis=1, kind="stable")[:, :TOP_K]  # [T, k]
    tw = np.take_along_axis(scores, idx, 1)
    m2 = tw.max(-1, keepdims=True)
    e2 = np.exp(tw - m2)
    w = (e2 / e2.sum(-1, keepdims=True)).astype(np.float32)
    return idx.astype(np.int64), w


def kernel(x, Wg, W1, b1, W2, b2):
    global LAST_EXEC_TIME_NS
    from concourse import bass_utils

    dt_w1, dt_w2a, dt_w2b = DT_W1, DT_W2A, DT_W2B
    dt_a, dt_y = DT_A, DT_Y
    orig_shape = x.shape
    x = np.asarray(x, dtype=np.float32)
    Wg = np.asarray(Wg, dtype=np.float32)
    W1 = np.asarray(W1, dtype=np.float32)
    b1 = np.asarray(b1, dtype=np.float32)
    W2 = np.asarray(W2, dtype=np.float32)
    b2 = np.asarray(b2, dtype=np.float32)
    xf = np.ascontiguousarray(x.reshape(-1, D))
    T = xf.shape[0]

    idx, w = _route(xf, Wg)

    # ---- slot assignment: rank experts by count, group ranks of N_CORES,
    # slot j capacity = max count in group j (aligned up to 4)
    counts = np.bincount(idx.reshape(-1), minlength=E)
    order_e = np.argsort(-counts, kind="stable")         # expert ids by rank
    exp_core = np.empty(E, np.int64)
    exp_slot = np.empty(E, np.int64)
    sizes = []
    for j in range(EPC):
        grp = order_e[j * N_CORES:(j + 1) * N_CORES]
        exp_core[grp] = np.arange(N_CORES)
        exp_slot[grp] = j
        sizes.append(max(4, int(-(-int(counts[grp].max()) // 4) * 4)))
    TT = sum(sizes)
    slot_off = np.concatenate([[0], np.cumsum(sizes)])   # token offsets per slot

    # ---- dispatch: sort assignments by expert, position within expert
    flat_e = idx.reshape(-1)                 # [T*k]
    flat_t = np.repeat(np.arange(T), TOP_K)
    sorder = np.argsort(flat_e, kind="stable")
    starts = np.zeros(E + 1, np.int64)
    starts[1:] = np.cumsum(counts)
    se = flat_e[sorder]
    pos = np.arange(TOP_K * T) - starts[se]
    core = exp_core[se]
    slot = exp_slot[se]
    tok = flat_t[sorder]

    # token id occupying each (core, slot, pos); padding -> token 0
    gidx = np.zeros((N_CORES, TT), np.int64)
    for c in range(N_CORES):
        msel = core == c
        gidx[c, slot_off[slot[msel]] + pos[msel]] = tok[msel]

    np_w1 = _np_dt(dt_w1)
    np_w2a = _np_dt(dt_w2a)
    np_w2b = _np_dt(dt_w2b)
    np_a = _np_dt(dt_a)
    xf_a = xf.astype(np_a, copy=False)
    # pre-tile weights partition-major:
    #   w1[e] = [p(=d_in%128), ht, dt(=d_in//128), hi]
    #   w2[e] = [p(=h_in%128), dt, ht(=h_in//128), di]  (split at dt=NDA)
    W1t = np.ascontiguousarray(
        (W1 * W1_SCALE).reshape(E, ND, 128, NH, 128)
        .transpose(0, 2, 3, 1, 4).astype(np_w1, copy=False))
    W2r = W2.reshape(E, NH, 128, ND, 128).transpose(0, 2, 3, 1, 4)
    W2ta = np.ascontiguousarray(
        (W2r[:, :, :NDA] * W2A_SCALE).astype(np_w2a, copy=False))
    W2tb = np.ascontiguousarray(
        (W2r[:, :, NDA:] * W2B_SCALE).astype(np_w2b, copy=False))
    b1t = np.ascontiguousarray(b1.reshape(E, NH, 128).transpose(2, 0, 1))
    b2t = np.ascontiguousarray(b2.reshape(E, ND, 128).transpose(2, 0, 1))

    # chunk-major (j, ci) column blocks; must match the device layout
    chunks = _slot_chunks(sizes)
    blocks = []          # (slot j, token offset within slot, width, colbase)
    colbase = 0
    for j in range(EPC):
        c0 = 0
        for cw in chunks[j]:
            blocks.append((j, c0, cw, colbase))
            c0 += cw
            colbase += cw

    in_maps = []
    for c in range(N_CORES):
        sl_experts = np.array(
            [order_e[j * N_CORES + c] for j in range(EPC)], np.int64)
        # xT: [128, ND*TT]; chunk block = [128, ND, cw] = x[tok, dt*128+p]
        xTc = np.zeros((128, ND * TT), np_a)
        for j, c0, cw, cb in blocks:
            blk = xf_a[gidx[c, slot_off[j] + c0: slot_off[j] + c0 + cw]]
            xTc[:, ND * cb: ND * (cb + cw)] = (
                blk.reshape(cw, ND, 128).transpose(2, 1, 0).reshape(128, ND * cw))
        in_maps.append({
            "xT": np.ascontiguousarray(xTc),
            "w1": W1t[sl_experts],
            "w2a": W2ta[sl_experts],
            "w2b": W2tb[sl_experts],
            "b1": np.ascontiguousarray(b1t[:, sl_experts]),
            "b2": np.ascontiguousarray(b2t[:, sl_experts]),
        })

    nc = _get_nc(sizes, dt_w1, dt_w2a, dt_w2b, dt_a, dt_y)
    trace = os.environ.get("MOE_TRACE", "0") == "1"
    res = bass_utils.run_bass_kernel_spmd(
        nc, in_maps, core_ids=list(range(N_CORES)), trace=trace,
    )
    LAST_EXEC_TIME_NS = res.exec_time_ns

    # ---- combine: y[t] += w * yT[core][:, chunk block][:, :, pos]
    Y = np.zeros((N_CORES, TT, D), np.float32)
    for c in range(N_CORES):
        yTc = np.asarray(res.results[c]["yT"], dtype=np.float32)
        for j, c0, cw, cb in blocks:
            blk = yTc[:, ND * cb: ND * (cb + cw)]
            Y[c, slot_off[j] + c0: slot_off[j] + c0 + cw] = (
                blk.reshape(128, ND, cw).transpose(2, 1, 0).reshape(cw, D))

    contrib = Y[core, slot_off[slot] + pos]  # [T*k, D] in sorted order
    inv = np.empty_like(sorder)
    inv[sorder] = np.arange(TOP_K * T)
    contrib = contrib[inv].reshape(T, TOP_K, D)
    y = (contrib * w[:, :, None]).sum(1).astype(np.float32)
    return y.reshape(orig_shape)
